# revision 1
# baseline (speedup 1.0000x reference)
"""Trainium2 Bass kernel for nn_CapLayerLP: box+cap+fairness QP via
primal-dual predictor-corrector interior point, 20 iterations.

Exploits G = [1^T; -I; I; f^T; -f^T]: the reduced KKT matrix is
diag(D) + w0*11^T + (wf1+wf2)*ff^T, solved by Woodbury with a 2x2 inner
system -> each iteration is O(n) vector work, no Cholesky, no Gram matmul.

Numerical scheme (validated in fp32 against the fp64 reference):
- residuals maintained analytically: r_k = phi_k * r_0, phi *= (1-alpha)
- scalar-constraint dual steps via exact 2x2-solve identities
  (sum(dx) = al/a, f.dx = be/b) to avoid catastrophic cancellation
- final rank-2 diag-weighted projection removes accumulated drift on the
  active global constraints, then clip to [0,1].

Sharding: batch is 1 and the per-iteration state is a single (128,38)
tile; the solve is latency-bound, so the kernel is replicated on all 8
cores (data-parallel over the only QP); core 0's output is returned.

Layout: n=1024 vectors live as (128,8) fp32 tiles (partition-major).
SZ (128,38) packs [sm(0:8)|sp(8:16)|s0,sf1,sf2(16:19)|zm|zp|z0,zf1,zf2];
scalar states are replicated across partitions so they can be used as
per-partition tensor_scalar operands. Cross-partition sum = ones-matmul
on PE (reduce+broadcast in one op); cross-partition max = PE transpose +
free-dim reduce + ones-broadcast.
"""
import numpy as np

import concourse.bass as bass
import concourse.bacc as bacc
import concourse.tile as tile
from concourse import mybir
from concourse.bass_utils import run_bass_kernel_spmd

AL = mybir.AluOpType
F32 = mybir.dt.float32
AX = mybir.AxisListType.X

N = 1024
P = 128
CO = N // P            # 8 cols per n-vector
V = 2 * CO             # 16: packed m+p vector block
NS = V + 3             # 19: s-block width (vec + 3 scalars)
C_CAP = 10.0
EPS = 1e-4
import os
ITERS = int(os.environ.get("KD_ITERS", "16"))
M_CONST = 2 * N + 3
CLAMP = 1e-30
TINY = 1e-12


def _build(nc: bass.Bass):
    x_d = nc.dram_tensor("x", [1, N], F32, kind="ExternalInput")
    f_d = nc.dram_tensor("ind", [N], mybir.dt.int32, kind="ExternalInput")
    ones_d = nc.dram_tensor("ones", [P, P], F32, kind="ExternalInput")
    ident_d = nc.dram_tensor("ident", [P, P], F32, kind="ExternalInput")
    out_d = nc.dram_tensor("out", [1, N], F32, kind="ExternalOutput")
    dbg_d = nc.dram_tensor("dbg", [P, 64], F32, kind="ExternalOutput")

    x_ap = x_d[:, :].rearrange("a (p c) -> a p c", p=P)[0]
    f_ap = f_d[:].rearrange("(p c) -> p c", p=P)
    o_ap = out_d[:, :].rearrange("a (p c) -> a p c", p=P)[0]

    with tile.TileContext(nc) as tc:
        with (
            tc.tile_pool(name="const", bufs=1) as cns,
            tc.tile_pool(name="state", bufs=1) as st,
            tc.tile_pool(name="scr", bufs=3) as sc,
            tc.tile_pool(name="psum", bufs=2, space="PSUM") as ps,
            tc.tile_pool(name="psum1", bufs=2, space="PSUM") as ps1,
            tc.tile_pool(name="psumq", bufs=2, space="PSUM") as psq,
        ):
            ONES = cns.tile([P, P], F32)
            IDENT = cns.tile([P, P], F32)
            nc.sync.dma_start(out=ONES[:, :], in_=ones_d[:, :])
            nc.sync.dma_start(out=IDENT[:, :], in_=ident_d[:, :])

            F8 = cns.tile([P, CO], F32)
            nc.gpsimd.dma_start(out=F8, in_=f_ap)  # int32 -> f32 cast
            OMF8 = cns.tile([P, CO], F32)          # 1 - f
            nc.vector.tensor_scalar(out=OMF8, in0=F8, scalar1=-1.0,
                                    scalar2=1.0, op0=AL.mult, op1=AL.add)

            XT = st.tile([P, CO], F32)      # x iterate
            nc.sync.dma_start(out=XT, in_=x_ap)
            RX0 = cns.tile([P, CO], F32)    # p + 1 = 1 - x_in
            nc.vector.tensor_scalar(out=RX0, in0=XT, scalar1=-1.0,
                                    scalar2=1.0, op0=AL.mult, op1=AL.add)
            nc.vector.memset(XT, 0.0)

            SZ = st.tile([P, 2 * NS], F32)
            nc.vector.memset(SZ, 1.0)
            PHI = st.tile([P, 1], F32)
            nc.vector.memset(PHI, 1.0)
            NPHI = st.tile([P, 1], F32)
            nc.vector.memset(NPHI, -1.0)

            # RF = [r00 | rf10 | rf20] = [1-C | -C*Nm/n | 1+C*Nm/n]
            # note hf2 = rf10 and hf1 = rf20 (reused by the end projection)
            RF = st.tile([P, 3], F32)
            facc = sc.tile([P, 1], F32, tag="facc")
            nc.vector.reduce_sum(facc, F8, axis=AX)
            NMp = ps.tile([P, 1], F32, tag="pscr")
            nc.tensor.matmul(NMp, ONES, facc)
            nc.vector.memset(RF[:, 0:1], 1.0 - C_CAP)
            nc.vector.tensor_scalar(out=RF[:, 1:2], in0=NMp,
                                    scalar1=-C_CAP / N, scalar2=None,
                                    op0=AL.mult)
            nc.vector.tensor_scalar(out=RF[:, 2:3], in0=NMp,
                                    scalar1=C_CAP / N, scalar2=1.0,
                                    op0=AL.mult, op1=AL.add)

            s_v = SZ[:, 0:V]            # [sm|sp]
            s_s = SZ[:, V:NS]           # [s0 sf1 sf2]
            z_v = SZ[:, NS:NS + V]
            z_s = SZ[:, NS + V:2 * NS]
            z_all = SZ[:, NS:2 * NS]
            s_all = SZ[:, 0:NS]

            def direction(DSZ, DX, rsz_v, rsz_s, R, W, DI, AINV, BINV,
                          VUSS, ApSd, DETI, RPs, tag):
                """Emit one Newton direction. DSZ layout mirrors SZ but
                holds [ds(0:19) | -dz(19:38)]. Returns albc psum tile of
                the step length (replicated) for this direction's ratio
                test? No: steplen is emitted separately."""
                t = tag
                # nt_s = -t_s = (rsz_s - z_s*rp_s) / s_s
                u_nt = sc.tile([P, 3], F32, tag=f"unt{t}")
                nc.gpsimd.tensor_tensor(out=u_nt, in0=z_s, in1=RPs,
                                        op=AL.mult)
                v_nt = sc.tile([P, 3], F32, tag=f"vnt{t}")
                nc.gpsimd.tensor_tensor(out=v_nt, in0=rsz_s, in1=u_nt,
                                        op=AL.subtract)
                NT = sc.tile([P, 3], F32, tag=f"nt{t}")
                nc.gpsimd.tensor_tensor(out=NT, in0=v_nt, in1=R[:, V:NS],
                                        op=AL.mult)
                NTDF = sc.tile([P, 1], F32, tag=f"ntdf{t}")
                nc.gpsimd.tensor_tensor(out=NTDF, in0=NT[:, 1:2],
                                        in1=NT[:, 2:3], op=AL.subtract)
                # tm = (zm*phi - rsz_m)/sm ; tp_pos = rsz_p/sp
                tmr = sc.tile([P, CO], F32, tag=f"tmr{t}")
                nc.vector.scalar_tensor_tensor(
                    out=tmr, in0=SZ[:, NS:NS + CO], scalar=PHI,
                    in1=rsz_v[:, 0:CO], op0=AL.mult, op1=AL.subtract)
                tm = sc.tile([P, CO], F32, tag=f"tm{t}")
                nc.vector.tensor_tensor(out=tm, in0=tmr, in1=R[:, 0:CO],
                                        op=AL.mult)
                tpp = sc.tile([P, CO], F32, tag=f"tpp{t}")
                nc.vector.tensor_tensor(out=tpp, in0=rsz_v[:, CO:V],
                                        in1=R[:, CO:V], op=AL.mult)
                # rhs = tm - phi*rx0 - tp_pos - tdf*f - t0 (t0 folded in y)
                A1 = sc.tile([P, CO], F32, tag=f"a1{t}")
                nc.vector.scalar_tensor_tensor(
                    out=A1, in0=RX0, scalar=NPHI, in1=tm,
                    op0=AL.mult, op1=AL.add)
                A2 = sc.tile([P, CO], F32, tag=f"a2{t}")
                nc.vector.tensor_tensor(out=A2, in0=A1, in1=tpp,
                                        op=AL.add)
                B1 = sc.tile([P, CO], F32, tag=f"b1{t}")
                nc.vector.scalar_tensor_tensor(
                    out=B1, in0=F8, scalar=NTDF, in1=A2,
                    op0=AL.mult, op1=AL.add)
                acc3 = sc.tile([P, 3], F32, tag=f"acc3{t}")
                Y = sc.tile([P, CO], F32, tag=f"y{t}")
                nc.vector.scalar_tensor_tensor(
                    out=Y, in0=B1, scalar=NT[:, 0:1], in1=DI,
                    op0=AL.add, op1=AL.mult, accum_out=acc3[:, 0:1])
                FYt = sc.tile([P, CO], F32, tag=f"fy{t}")
                nc.vector.scalar_tensor_tensor(
                    out=FYt, in0=Y, scalar=1.0, in1=F8,
                    op0=AL.bypass, op1=AL.mult, accum_out=acc3[:, 1:2])
                YMF = sc.tile([P, CO], F32, tag=f"ymf{t}")
                nc.vector.scalar_tensor_tensor(
                    out=YMF, in0=Y, scalar=1.0, in1=OMF8,
                    op0=AL.bypass, op1=AL.mult, accum_out=acc3[:, 2:3])
                S12 = ps.tile([P, 3], F32, tag="pscr")
                nc.tensor.matmul(S12, ONES, acc3)  # [S1|S2|S1m2] replicated
                AB2 = sc.tile([P, 2], F32, tag=f"ab2{t}")
                q2 = sc.tile([P, 1], F32, tag=f"q2{t}")
                nc.vector.tensor_tensor(out=q2, in0=VUSS[:, 0:1],
                                        in1=S12[:, 2:3], op=AL.mult)
                nc.vector.tensor_scalar(out=AB2[:, 0:1], in0=BINV,
                                        scalar1=S12[:, 0:1], scalar2=q2,
                                        op0=AL.mult, op1=AL.add)
                nc.vector.tensor_scalar(out=AB2[:, 1:2], in0=ApSd,
                                        scalar1=S12[:, 1:2], scalar2=q2,
                                        op0=AL.mult, op1=AL.subtract)
                albe = sc.tile([P, 2], F32, tag=f"albe{t}")
                nc.vector.tensor_scalar(out=albe, in0=AB2, scalar1=DETI,
                                        scalar2=None, op0=AL.mult)
                c8 = sc.tile([P, CO], F32, tag=f"c8{t}")
                nc.vector.tensor_scalar(out=c8, in0=F8,
                                        scalar1=albe[:, 1:2],
                                        scalar2=albe[:, 0:1],
                                        op0=AL.mult, op1=AL.add)
                m1 = sc.tile([P, CO], F32, tag=f"m1{t}")
                nc.vector.tensor_tensor(out=m1, in0=DI, in1=c8, op=AL.mult)
                nc.vector.tensor_tensor(out=DX, in0=Y, in1=m1,
                                        op=AL.subtract)
                # scalar steps via exact identities
                SFX = sc.tile([P, 3], F32, tag=f"sfx{t}")
                nc.vector.tensor_tensor(out=SFX[:, 0:1], in0=AINV,
                                        in1=albe[:, 0:1], op=AL.mult)
                nc.vector.tensor_tensor(out=SFX[:, 1:2], in0=BINV,
                                        in1=albe[:, 1:2], op=AL.mult)
                nc.vector.tensor_scalar(out=SFX[:, 2:3], in0=SFX[:, 1:2],
                                        scalar1=-1.0, scalar2=None,
                                        op0=AL.mult)
                nc.vector.scalar_tensor_tensor(
                    out=DSZ[:, V:NS], in0=RPs, scalar=-1.0, in1=SFX,
                    op0=AL.mult, op1=AL.subtract)  # ds_s = -rp_s - SFX
                ADD3 = sc.tile([P, 3], F32, tag=f"ad3{t}")
                nc.vector.tensor_copy(ADD3[:, 0:1], albe[:, 0:1])
                nc.vector.tensor_tensor(out=ADD3[:, 1:3], in0=W[:, V + 1:NS],
                                        in1=SFX[:, 1:3], op=AL.mult)
                # ndz_s = nt_s - ADD3
                nc.vector.tensor_tensor(out=DSZ[:, NS + V:2 * NS], in0=NT,
                                        in1=ADD3, op=AL.subtract)
                # vector ds / ndz
                nc.vector.tensor_scalar(out=DSZ[:, 0:CO], in0=DX,
                                        scalar1=NPHI, scalar2=None,
                                        op0=AL.add)           # dsm
                nc.scalar.mul(DSZ[:, CO:V], DX, -1.0)     # dsp
                uv = sc.tile([P, V], F32, tag=f"uv{t}")
                nc.vector.tensor_tensor(out=uv, in0=z_v, in1=DSZ[:, 0:V],
                                        op=AL.mult)
                vv = sc.tile([P, V], F32, tag=f"vv{t}")
                nc.vector.tensor_tensor(out=vv, in0=uv, in1=rsz_v,
                                        op=AL.add)
                nc.vector.tensor_tensor(out=DSZ[:, NS:NS + V], in0=vv,
                                        in1=R[:, 0:V], op=AL.mult)  # -dz_v

            def steplen(DSZ, R, tag):
                """Return psum (128,1) tile holding 1/max(1, qmax)."""
                t = tag
                Q = sc.tile([P, 2 * NS], F32, tag=f"q{t}")
                nc.vector.scalar_tensor_tensor(
                    out=Q[:, 0:NS], in0=DSZ[:, 0:NS], scalar=-1.0,
                    in1=R[:, 0:NS], op0=AL.mult, op1=AL.mult)  # -ds/s
                nc.vector.tensor_tensor(out=Q[:, NS:2 * NS],
                                        in0=DSZ[:, NS:2 * NS],
                                        in1=R[:, NS:2 * NS],
                                        op=AL.mult)            # ndz/z
                qp = sc.tile([P, 1], F32, tag=f"qp{t}")
                nc.vector.reduce_max(qp, Q, axis=AX)
                qrow = psq.tile([1, P], F32, tag="qrow")
                nc.tensor.transpose(qrow, qp, IDENT)
                qm = sc.tile([1, 1], F32, tag=f"qm{t}")
                nc.vector.reduce_max(qm, qrow, axis=AX)
                qc = sc.tile([1, 1], F32, tag=f"qc{t}")
                nc.vector.tensor_scalar(out=qc, in0=qm, scalar1=1.0,
                                        scalar2=None, op0=AL.max)
                qr = sc.tile([1, 1], F32, tag=f"qr{t}")
                nc.vector.reciprocal(qr, qc)
                albc = ps1.tile([P, 1], F32, tag="albc")
                nc.tensor.matmul(albc, ONES[0:1, :], qr)
                return albc

            for it in range(ITERS):
                # ---- stage A: iteration-level quantities ----
                R = sc.tile([P, 2 * NS], F32, tag="R")
                nc.vector.reciprocal(R, SZ)
                W = sc.tile([P, NS], F32, tag="W")
                nc.vector.tensor_tensor(out=W, in0=z_all, in1=R[:, 0:NS],
                                        op=AL.mult)
                DI = sc.tile([P, CO], F32, tag="DI")
                Dt = sc.tile([P, CO], F32, tag="Dt")
                nc.vector.scalar_tensor_tensor(
                    out=Dt, in0=W[:, 0:CO], scalar=EPS, in1=W[:, CO:V],
                    op0=AL.add, op1=AL.add)
                nc.vector.reciprocal(DI, Dt)
                acc2 = sc.tile([P, 3], F32, tag="acc2")  # [Sv|Sd|mac]
                DIF = sc.tile([P, CO], F32, tag="DIF")
                nc.vector.scalar_tensor_tensor(
                    out=DIF, in0=DI, scalar=1.0, in1=F8,
                    op0=AL.bypass, op1=AL.mult, accum_out=acc2[:, 0:1])
                DIMF = sc.tile([P, CO], F32, tag="DIMF")
                nc.vector.scalar_tensor_tensor(
                    out=DIMF, in0=DI, scalar=1.0, in1=OMF8,
                    op0=AL.bypass, op1=AL.mult, accum_out=acc2[:, 1:2])
                SZPv = sc.tile([P, V], F32, tag="SZPv")
                nc.vector.scalar_tensor_tensor(
                    out=SZPv, in0=s_v, scalar=1.0, in1=z_v,
                    op0=AL.bypass, op1=AL.mult, accum_out=acc2[:, 2:3])
                VUS = ps.tile([P, 3], F32, tag="pscr")  # [Sv|Sd|Mv]
                nc.tensor.matmul(VUS, ONES, acc2)
                VUSS = sc.tile([P, 3], F32, tag="VUSS")
                nc.scalar.copy(VUSS, VUS)
                AINV = sc.tile([P, 1], F32, tag="AINV")  # s0/z0
                nc.vector.tensor_tensor(out=AINV, in0=SZ[:, V:V + 1],
                                        in1=R[:, NS + V:NS + V + 1],
                                        op=AL.mult)
                Bt = sc.tile([P, 1], F32, tag="Bt")
                nc.vector.tensor_tensor(out=Bt, in0=W[:, V + 1:V + 2],
                                        in1=W[:, V + 2:V + 3], op=AL.add)
                BINV = sc.tile([P, 1], F32, tag="BINV")
                nc.vector.reciprocal(BINV, Bt)
                # det = ainv*(binv+Sv) + binv*(Sv+Sd) + Sv*Sd  (all +)
                SuT = sc.tile([P, 1], F32, tag="SuT")
                nc.vector.tensor_tensor(out=SuT, in0=VUSS[:, 0:1],
                                        in1=VUSS[:, 1:2], op=AL.add)
                M22t = sc.tile([P, 1], F32, tag="M22t")
                nc.vector.tensor_tensor(out=M22t, in0=BINV,
                                        in1=VUSS[:, 0:1], op=AL.add)
                qa = sc.tile([P, 1], F32, tag="qa")
                nc.vector.tensor_tensor(out=qa, in0=BINV, in1=SuT,
                                        op=AL.mult)
                qb = sc.tile([P, 1], F32, tag="qb")
                nc.vector.tensor_scalar(out=qb, in0=VUSS[:, 0:1],
                                        scalar1=VUSS[:, 1:2], scalar2=qa,
                                        op0=AL.mult, op1=AL.add)
                DETt = sc.tile([P, 1], F32, tag="DETt")
                nc.vector.tensor_scalar(out=DETt, in0=AINV, scalar1=M22t,
                                        scalar2=qb, op0=AL.mult, op1=AL.add)
                DETI = sc.tile([P, 1], F32, tag="DETI")
                nc.vector.reciprocal(DETI, DETt)
                ApSd = sc.tile([P, 1], F32, tag="ApSd")
                nc.vector.tensor_tensor(out=ApSd, in0=AINV,
                                        in1=VUSS[:, 1:2], op=AL.add)
                RPs = sc.tile([P, 3], F32, tag="RPs")
                nc.vector.tensor_scalar(out=RPs, in0=RF, scalar1=PHI,
                                        scalar2=None, op0=AL.mult)

                # ---- mu scalar part (vec part rides in acc2 col2) ----
                SZPs = sc.tile([P, 3], F32, tag="SZPs")
                nc.vector.tensor_tensor(out=SZPs, in0=s_s, in1=z_s,
                                        op=AL.mult)
                msc = sc.tile([P, 1], F32, tag="msc")
                nc.vector.reduce_sum(msc, SZPs, axis=AX)
                MUm = sc.tile([P, 1], F32, tag="MUm")
                nc.vector.tensor_tensor(out=MUm, in0=msc,
                                        in1=VUSS[:, 2:3], op=AL.add)

                # ---- affine direction ----
                DSZa = sc.tile([P, 2 * NS], F32, tag="DSZa")
                DXa = sc.tile([P, CO], F32, tag="DXa")
                direction(DSZa, DXa, SZPv, SZPs, R, W, DI, AINV, BINV,
                          VUSS, ApSd, DETI, RPs, "a")
                # alpha-independent corrector products: emitted before
                # steplen so the scheduler fills the PE round-trip gap
                pqv = sc.tile([P, V], F32, tag="pqv")
                nc.vector.scalar_tensor_tensor(
                    out=pqv, in0=DSZa[:, 0:V], scalar=-1.0,
                    in1=DSZa[:, NS:NS + V], op0=AL.mult, op1=AL.mult)
                pqs = sc.tile([P, 3], F32, tag="pqs")
                nc.vector.scalar_tensor_tensor(
                    out=pqs, in0=DSZa[:, V:NS], scalar=-1.0,
                    in1=DSZa[:, NS + V:2 * NS], op0=AL.mult, op1=AL.mult)
                aaff = steplen(DSZa, R, "a")  # psum (128,1)
                naff = sc.tile([P, 1], F32, tag="naff")
                nc.scalar.mul(naff, aaff, -1.0)

                # ---- mu_aff ----
                st19 = sc.tile([P, NS], F32, tag="st19")
                nc.vector.scalar_tensor_tensor(
                    out=st19, in0=DSZa[:, 0:NS], scalar=aaff, in1=s_all,
                    op0=AL.mult, op1=AL.add)
                zt19 = sc.tile([P, NS], F32, tag="zt19")
                nc.vector.scalar_tensor_tensor(
                    out=zt19, in0=DSZa[:, NS:2 * NS], scalar=naff,
                    in1=z_all, op0=AL.mult, op1=AL.add)
                mac2 = sc.tile([P, 1], F32, tag="mac2")
                pv = sc.tile([P, V], F32, tag="pv")
                nc.vector.scalar_tensor_tensor(
                    out=pv, in0=st19[:, 0:V], scalar=1.0,
                    in1=zt19[:, 0:V], op0=AL.bypass, op1=AL.mult,
                    accum_out=mac2)
                pss = sc.tile([P, 3], F32, tag="pss")
                nc.vector.tensor_tensor(out=pss, in0=st19[:, V:NS],
                                        in1=zt19[:, V:NS], op=AL.mult)
                msc2 = sc.tile([P, 1], F32, tag="msc2")
                nc.vector.reduce_sum(msc2, pss, axis=AX)
                MAP = ps.tile([P, 1], F32, tag="pscr")
                nc.tensor.matmul(MAP, ONES, mac2)
                MAm = sc.tile([P, 1], F32, tag="MAm")
                nc.vector.tensor_scalar(out=MAm, in0=msc2, scalar1=MAP,
                                        scalar2=None, op0=AL.add)
                # smu = (mu_aff/mu)^3 * mu = MAm^3/(MUm^2 * m) ... via ratio
                mui = sc.tile([P, 1], F32, tag="mui")
                nc.vector.reciprocal(mui, MUm)
                rat = sc.tile([P, 1], F32, tag="rat")
                nc.vector.tensor_scalar(out=rat, in0=MAm, scalar1=mui,
                                        scalar2=None, op0=AL.mult)
                r2 = sc.tile([P, 1], F32, tag="r2")
                nc.vector.tensor_scalar(out=r2, in0=rat, scalar1=rat,
                                        scalar2=None, op0=AL.mult)
                r3 = sc.tile([P, 1], F32, tag="r3")
                nc.vector.tensor_scalar(out=r3, in0=r2, scalar1=rat,
                                        scalar2=None, op0=AL.mult)
                NSMU = sc.tile([P, 1], F32, tag="NSMU")
                nc.vector.scalar_tensor_tensor(
                    out=NSMU, in0=r3, scalar=-1.0 / M_CONST, in1=MUm,
                    op0=AL.mult, op1=AL.mult)  # -sigma*mu

                # ---- corrector rsz ----
                RCv = sc.tile([P, V], F32, tag="RCv")
                nc.vector.scalar_tensor_tensor(
                    out=RCv, in0=pqv, scalar=NSMU, in1=SZPv,
                    op0=AL.add, op1=AL.add)
                RCs = sc.tile([P, 3], F32, tag="RCs")
                nc.vector.scalar_tensor_tensor(
                    out=RCs, in0=pqs, scalar=NSMU, in1=SZPs,
                    op0=AL.add, op1=AL.add)

                # ---- corrector direction + step ----
                DSZc = sc.tile([P, 2 * NS], F32, tag="DSZc")
                DXc = sc.tile([P, CO], F32, tag="DXc")
                direction(DSZc, DXc, RCv, RCs, R, W, DI, AINV, BINV,
                          VUSS, ApSd, DETI, RPs, "c")
                acor = steplen(DSZc, R, "c")
                ALC = sc.tile([P, 1], F32, tag="ALC")
                nc.vector.tensor_scalar(out=ALC, in0=acor, scalar1=0.99,
                                        scalar2=None, op0=AL.mult)
                NALC = sc.tile([P, 1], F32, tag="NALC")
                nc.vector.tensor_scalar(out=NALC, in0=acor, scalar1=-0.99,
                                        scalar2=None, op0=AL.mult)
                OneM = sc.tile([P, 1], F32, tag="OneM")
                nc.vector.tensor_scalar(out=OneM, in0=acor, scalar1=-0.99,
                                        scalar2=1.0, op0=AL.mult,
                                        op1=AL.add)

                # ---- updates ----
                nc.vector.scalar_tensor_tensor(
                    out=XT, in0=DXc, scalar=ALC, in1=XT,
                    op0=AL.mult, op1=AL.add)
                nc.vector.scalar_tensor_tensor(
                    out=s_all, in0=DSZc[:, 0:NS], scalar=ALC, in1=s_all,
                    op0=AL.mult, op1=AL.add)
                nc.vector.scalar_tensor_tensor(
                    out=z_all, in0=DSZc[:, NS:2 * NS], scalar=NALC,
                    in1=z_all, op0=AL.mult, op1=AL.add)
                nc.vector.tensor_scalar(out=SZ, in0=SZ, scalar1=CLAMP,
                                        scalar2=None, op0=AL.max)
                nc.vector.tensor_tensor(out=PHI, in0=PHI, in1=OneM,
                                        op=AL.mult)
                nc.vector.tensor_scalar(out=NPHI, in0=PHI, scalar1=-1.0,
                                        scalar2=None, op0=AL.mult)

            # ---- end projection ----
            XTpre = st.tile([P, CO], F32)
            nc.vector.tensor_copy(XTpre, XT)
            accF = sc.tile([P, 2], F32, tag="accF")
            fxv = sc.tile([P, CO], F32, tag="fxv")
            nc.vector.scalar_tensor_tensor(
                out=fxv, in0=XT, scalar=1.0, in1=F8,
                op0=AL.bypass, op1=AL.mult, accum_out=accF[:, 1:2])
            nc.vector.reduce_sum(accF[:, 0:1], XT, axis=AX)
            SXF = ps.tile([P, 2], F32, tag="pscr")  # [Sx|Fx]
            nc.tensor.matmul(SXF, ONES, accF)

            R2 = sc.tile([P, 2 * NS], F32, tag="R")
            nc.vector.reciprocal(R2, SZ)
            W2 = sc.tile([P, NS], F32, tag="W")
            nc.vector.tensor_tensor(out=W2, in0=z_all, in1=R2[:, 0:NS],
                                    op=AL.mult)
            D2 = sc.tile([P, CO], F32, tag="Dt")
            nc.vector.scalar_tensor_tensor(
                out=D2, in0=W2[:, 0:CO], scalar=EPS, in1=W2[:, CO:V],
                op0=AL.add, op1=AL.add)
            DI2 = sc.tile([P, CO], F32, tag="DI")
            nc.vector.reciprocal(DI2, D2)
            nc.vector.tensor_scalar(out=DI2, in0=DI2, scalar1=1e-4,
                                    scalar2=None, op0=AL.max)
            acc2f = sc.tile([P, 2], F32, tag="acc2")
            DIF2 = sc.tile([P, CO], F32, tag="DIF")
            nc.vector.scalar_tensor_tensor(
                out=DIF2, in0=DI2, scalar=1.0, in1=F8,
                op0=AL.bypass, op1=AL.mult, accum_out=acc2f[:, 0:1])
            nc.vector.reduce_sum(acc2f[:, 1:2], DI2, axis=AX)
            VUS2p = ps.tile([P, 2], F32, tag="pscr")  # [Sv|Su]
            nc.tensor.matmul(VUS2p, ONES, acc2f)
            VUS2 = sc.tile([P, 2], F32, tag="VUS2")
            nc.vector.tensor_copy(VUS2, VUS2p)

            GT3 = sc.tile([P, 3], F32, tag="GT3")  # [g0 gf1 gf2]
            nc.vector.tensor_tensor(out=GT3, in0=z_s, in1=s_s, op=AL.is_gt)
            d0 = sc.tile([P, 1], F32, tag="d0")
            nc.vector.scalar_tensor_tensor(
                out=d0, in0=SXF[:, 0:1], scalar=-C_CAP, in1=s_s[:, 0:1],
                op0=AL.add, op1=AL.add)
            ta = sc.tile([P, 1], F32, tag="ta")
            nc.vector.tensor_tensor(out=ta, in0=SXF[:, 1:2],
                                    in1=s_s[:, 1:2], op=AL.add)
            dfa = sc.tile([P, 1], F32, tag="dfa")
            nc.vector.tensor_tensor(out=dfa, in0=ta, in1=RF[:, 2:3],
                                    op=AL.subtract)
            tb = sc.tile([P, 1], F32, tag="tb")
            nc.vector.tensor_tensor(out=tb, in0=s_s[:, 2:3],
                                    in1=SXF[:, 1:2], op=AL.subtract)
            dfb = sc.tile([P, 1], F32, tag="dfb")
            nc.vector.tensor_tensor(out=dfb, in0=tb, in1=RF[:, 1:2],
                                    op=AL.subtract)
            ua = sc.tile([P, 1], F32, tag="ua")
            nc.vector.tensor_tensor(out=ua, in0=GT3[:, 1:2], in1=dfa,
                                    op=AL.mult)
            ub = sc.tile([P, 1], F32, tag="ub")
            nc.vector.tensor_tensor(out=ub, in0=GT3[:, 2:3], in1=dfb,
                                    op=AL.mult)
            df = sc.tile([P, 1], F32, tag="df")
            nc.vector.tensor_tensor(out=df, in0=ua, in1=ub,
                                    op=AL.subtract)
            gf = sc.tile([P, 1], F32, tag="gf")
            nc.vector.tensor_tensor(out=gf, in0=GT3[:, 1:2],
                                    in1=GT3[:, 2:3], op=AL.max)
            Sd = sc.tile([P, 1], F32, tag="Sd")
            nc.vector.tensor_tensor(out=Sd, in0=VUS2[:, 1:2],
                                    in1=VUS2[:, 0:1], op=AL.subtract)
            gdf = sc.tile([P, 1], F32, tag="gdf")
            nc.vector.tensor_tensor(out=gdf, in0=gf, in1=df, op=AL.mult)
            num0 = sc.tile([P, 1], F32, tag="num0")
            nc.vector.tensor_tensor(out=num0, in0=d0, in1=gdf,
                                    op=AL.subtract)
            gsv = sc.tile([P, 1], F32, tag="gsv")
            nc.vector.tensor_tensor(out=gsv, in0=gf, in1=VUS2[:, 0:1],
                                    op=AL.mult)
            den0 = sc.tile([P, 1], F32, tag="den0")
            nc.vector.tensor_tensor(out=den0, in0=VUS2[:, 1:2], in1=gsv,
                                    op=AL.subtract)
            dd = sc.tile([P, 1], F32, tag="dd")
            nc.vector.scalar_tensor_tensor(
                out=dd, in0=den0, scalar=1.0, in1=den0,
                op0=AL.bypass, op1=AL.mult)
            ddt = sc.tile([P, 1], F32, tag="ddt")
            nc.vector.tensor_scalar(out=ddt, in0=dd, scalar1=TINY,
                                    scalar2=None, op0=AL.add)
            rdd = sc.tile([P, 1], F32, tag="rdd")
            nc.vector.reciprocal(rdd, ddt)
            v0a = sc.tile([P, 1], F32, tag="v0a")
            nc.vector.tensor_tensor(out=v0a, in0=num0, in1=den0,
                                    op=AL.mult)
            v0b = sc.tile([P, 1], F32, tag="v0b")
            nc.vector.tensor_tensor(out=v0b, in0=v0a, in1=rdd,
                                    op=AL.mult)
            v0 = sc.tile([P, 1], F32, tag="v0")
            nc.vector.tensor_tensor(out=v0, in0=GT3[:, 0:1], in1=v0b,
                                    op=AL.mult)
            sv2 = sc.tile([P, 1], F32, tag="sv2")
            nc.vector.scalar_tensor_tensor(
                out=sv2, in0=VUS2[:, 0:1], scalar=1.0, in1=VUS2[:, 0:1],
                op0=AL.bypass, op1=AL.mult)
            sv2t = sc.tile([P, 1], F32, tag="sv2t")
            nc.vector.tensor_scalar(out=sv2t, in0=sv2, scalar1=TINY,
                                    scalar2=None, op0=AL.add)
            rsv = sc.tile([P, 1], F32, tag="rsv")
            nc.vector.reciprocal(rsv, sv2t)
            u1 = sc.tile([P, 1], F32, tag="u1")
            nc.vector.tensor_tensor(out=u1, in0=df, in1=VUS2[:, 0:1],
                                    op=AL.mult)
            v1a = sc.tile([P, 1], F32, tag="v1a")
            nc.vector.tensor_tensor(out=v1a, in0=u1, in1=rsv, op=AL.mult)
            w1 = sc.tile([P, 1], F32, tag="w1")
            nc.vector.tensor_tensor(out=w1, in0=gf, in1=v1a, op=AL.mult)
            omgf = sc.tile([P, 1], F32, tag="omgf")
            nc.vector.tensor_scalar(out=omgf, in0=gf, scalar1=-1.0,
                                    scalar2=1.0, op0=AL.mult, op1=AL.add)
            w3 = sc.tile([P, 1], F32, tag="w3")
            nc.vector.tensor_tensor(out=w3, in0=omgf, in1=v0, op=AL.mult)
            v1 = sc.tile([P, 1], F32, tag="v1")
            nc.vector.tensor_tensor(out=v1, in0=w1, in1=w3, op=AL.add)
            bee = sc.tile([P, 1], F32, tag="bee")
            nc.vector.tensor_tensor(out=bee, in0=v1, in1=v0,
                                    op=AL.subtract)
            corr = sc.tile([P, CO], F32, tag="corr")
            nc.vector.tensor_scalar(out=corr, in0=F8, scalar1=bee,
                                    scalar2=v0, op0=AL.mult, op1=AL.add)
            mcor = sc.tile([P, CO], F32, tag="mcor")
            nc.vector.tensor_tensor(out=mcor, in0=DI2, in1=corr,
                                    op=AL.mult)
            nc.vector.tensor_tensor(out=XT, in0=XT, in1=mcor,
                                    op=AL.subtract)
            nc.vector.tensor_scalar(out=XT, in0=XT, scalar1=0.0,
                                    scalar2=1.0, op0=AL.max, op1=AL.min)

            DBG = st.tile([P, 64], F32)
            nc.vector.tensor_copy(DBG[:, 0:CO], F8)
            nc.vector.tensor_copy(DBG[:, 8:16], RX0)
            nc.vector.tensor_copy(DBG[:, 16:54], SZ)
            nc.vector.tensor_copy(DBG[:, 54:62], XTpre)
            nc.vector.tensor_copy(DBG[:, 62:63], PHI)
            nc.vector.tensor_copy(DBG[:, 63:64], RF[:, 1:2])
            nc.sync.dma_start(out=dbg_d[:, :], in_=DBG)
            nc.sync.dma_start(out=o_ap, in_=XT)

    return nc


_CACHE: dict = {}


def _get_nc():
    if "nc" not in _CACHE:
        nc = bacc.Bacc(None, target_bir_lowering=False)
        _build(nc)
        nc.finalize()
        _CACHE["nc"] = nc
    return _CACHE["nc"]


def kernel(x: np.ndarray, indices_male: np.ndarray) -> np.ndarray:
    nc = _get_nc()
    base = {
        "x": np.ascontiguousarray(x, dtype=np.float32),
        "ind": np.ascontiguousarray(indices_male, dtype=np.int32),
        "ones": np.ones((P, P), dtype=np.float32),
        "ident": np.eye(P, dtype=np.float32),
    }
    in_maps = [dict(base) for _ in range(8)]
    res = run_bass_kernel_spmd(nc, in_maps, core_ids=list(range(8)))
    if os.environ.get("KD_DBG"):
        kernel.dbg = np.asarray(res.results[0]["dbg"])  # type: ignore
    return np.asarray(res.results[0]["out"], dtype=np.float32)


if __name__ == "__main__":
    rng = np.random.default_rng(0)
    x = rng.standard_normal((1, N)).astype(np.float32)
    f = (np.arange(N) % 2).astype(np.int32)
    out = kernel(x, f)
    print("out", out.shape, out.dtype, out[0, :6], out.sum())



# revision 2
# speedup vs baseline: 4.0999x; 4.0999x over previous
"""Trainium2 Bass kernel v2 for nn_CapLayerLP — instruction-latency
optimized predictor-corrector interior point (same math as baseline).

Changes vs baseline (359.7us):
- steplen: fused tensor_tensor_reduce (elementwise*scale + max-reduce with
  chained init) — 2 DVE ops instead of 6.
- mu_aff: computed from accumulator columns (sum z*ds, sum s*dz, sum ds*dz)
  via one ONES-matmul; sigma/nsmu math on partition 0 (1,1) tiles; only
  nsmu is broadcast — the affine step length never leaves partition 0.
- 2x2 Woodbury solve refactored to adjugate form (M22*S1 - Sv*S2 etc.)
  with DETI folded into DI (DIDETI) — fewer serial ops after the S12
  reduction matmul.
- scalar-constraint lanes + per-partition scalar products offloaded to
  GpSimd/Activation engines, emitted in dependency-urgency order.
- ONES via memset and IDENT via affine_select (no DMA); no debug output.
"""
import numpy as np
import os

import concourse.bass as bass
import concourse.bacc as bacc
import concourse.tile as tile
from concourse import mybir
from concourse.bass_utils import run_bass_kernel_spmd

AL = mybir.AluOpType
F32 = mybir.dt.float32
AX = mybir.AxisListType.X
ACT = mybir.ActivationFunctionType

N = 1024
P = 128
CO = N // P            # 8
V = 2 * CO             # 16
NS = V + 3             # 19
C_CAP = 10.0
EPS = 1e-4
ITERS = int(os.environ.get("KD_ITERS", "16"))
M_CONST = 2 * N + 3
CLAMP = 1e-30
TINY = 1e-12
NEGINF = -3.0e38


def _build(nc: bass.Bass):
    x_d = nc.dram_tensor("x", [1, N], F32, kind="ExternalInput")
    f_d = nc.dram_tensor("ind", [N], mybir.dt.int32, kind="ExternalInput")
    out_d = nc.dram_tensor("out", [1, N], F32, kind="ExternalOutput")

    x_ap = x_d[:, :].rearrange("a (p c) -> a p c", p=P)[0]
    f_ap = f_d[:].rearrange("(p c) -> p c", p=P)
    o_ap = out_d[:, :].rearrange("a (p c) -> a p c", p=P)[0]

    with tile.TileContext(nc) as tc:
        with (
            tc.tile_pool(name="const", bufs=1) as cns,
            tc.tile_pool(name="state", bufs=1) as st,
            tc.tile_pool(name="scr", bufs=2) as sc,
            tc.tile_pool(name="psum", bufs=1, space="PSUM") as ps,
            tc.tile_pool(name="psumq", bufs=1, space="PSUM") as psq,
        ):
            v = nc.vector
            g = nc.gpsimd
            a = nc.scalar
            pe = nc.tensor

            # ---------------- constants / inputs ----------------
            ONES = cns.tile([P, P], F32)
            g.memset(ONES, 1.0)
            IDENT = cns.tile([P, P], F32)
            # IDENT[p, y] = (p - y == 0) ? 1.0 : 0.0
            g.affine_select(out=IDENT, in_=ONES,
                            compare_op=AL.is_equal, fill=0.0, base=0,
                            pattern=[[-1, P]], channel_multiplier=1)

            F8 = cns.tile([P, CO], F32)
            nc.gpsimd.dma_start(out=F8, in_=f_ap)  # int32 -> f32 cast
            XIN = sc.tile([P, CO], F32, tag="xin")
            nc.sync.dma_start(out=XIN, in_=x_ap)

            RX0 = cns.tile([P, CO], F32)    # 1 - x_in
            v.tensor_scalar(out=RX0, in0=XIN, scalar1=-1.0, scalar2=1.0,
                            op0=AL.mult, op1=AL.add)
            XT = st.tile([P, CO], F32)
            v.memset(XT, 0.0)
            SZ = st.tile([P, 2 * NS], F32)
            v.memset(SZ, 1.0)
            PHI = st.tile([P, 1], F32)
            v.memset(PHI, 1.0)
            NPHI = st.tile([P, 1], F32)
            v.memset(NPHI, -1.0)

            facc = sc.tile([P, 1], F32, tag="facc")
            v.reduce_sum(facc, F8, axis=AX)
            NMp = ps.tile([P, 1], F32, tag="b0")
            pe.matmul(NMp, ONES, facc)
            RF = cns.tile([P, 3], F32)      # [1-C | -C*Nm/n | 1+C*Nm/n]
            v.memset(RF[:, 0:1], 1.0 - C_CAP)
            v.tensor_scalar(out=RF[:, 1:2], in0=NMp, scalar1=-C_CAP / N,
                            scalar2=None, op0=AL.mult)
            v.tensor_scalar(out=RF[:, 2:3], in0=NMp, scalar1=C_CAP / N,
                            scalar2=1.0, op0=AL.mult, op1=AL.add)

            SGN38 = cns.tile([P, 2 * NS], F32)   # [-1 x19 | +1 x19]
            v.memset(SGN38[:, 0:NS], -1.0)
            v.memset(SGN38[:, NS:2 * NS], 1.0)
            ONESB = cns.tile([1, P], mybir.dt.bfloat16)
            v.memset(ONESB, 1.0)
            CA3 = cns.tile([1, 3], F32)     # 0.99 * [1, -1, -1]
            v.memset(CA3[0:1, 0:1], 0.99)
            v.memset(CA3[0:1, 1:3], -0.99)
            CB3 = cns.tile([1, 3], F32)     # [0, 0, 1]
            v.memset(CB3[0:1, 0:2], 0.0)
            v.memset(CB3[0:1, 2:3], 1.0)

            s_v = SZ[:, 0:V]
            s_s = SZ[:, V:NS]
            z_v = SZ[:, NS:NS + V]
            z_s = SZ[:, NS + V:2 * NS]

            def direction(t, rsz_v, rsz_s, R, RS38, RPs, NTt, DIB,
                          DIDETI, VUSS, M22, M11, PP3, PN3, W16, Ws3,
                          PRX, DSZ, DXt, mu_acc, mid_cb=None):
                affine = mu_acc
                """Emit one Newton direction into DSZ=[ds|ndz] and DXt.
                mid_cb() emits filler ops during the S12 PE round trip.
                Returns (QP2, MAC, PQ16, PQS)."""
                # gpsimd: scalar-lane NT
                if affine:
                    v_nt = sc.tile([P, 3], F32, tag=f"vnt{t}")
                    g.tensor_tensor(out=v_nt, in0=s_s, in1=RPs,
                                    op=AL.subtract)
                    g.tensor_tensor(out=NTt, in0=Ws3, in1=v_nt, op=AL.mult)
                else:
                    u_nt = sc.tile([P, 3], F32, tag=f"unt{t}")
                    g.tensor_tensor(out=u_nt, in0=z_s, in1=RPs, op=AL.mult)
                    v_nt = sc.tile([P, 3], F32, tag=f"vnt{t}")
                    g.tensor_tensor(out=v_nt, in0=rsz_s, in1=u_nt,
                                    op=AL.subtract)
                    g.tensor_tensor(out=NTt, in0=v_nt, in1=R[:, V:NS],
                                    op=AL.mult)
                NTDF = sc.tile([P, 1], F32, tag=f"ntdf{t}")
                g.tensor_tensor(out=NTDF, in0=NTt[:, 1:2], in1=NTt[:, 2:3],
                                op=AL.subtract)
                # vector main chain
                tmr = sc.tile([P, CO], F32, tag=f"tmr{t}")
                v.scalar_tensor_tensor(out=tmr, in0=z_v[:, 0:CO],
                                       scalar=PHI, in1=rsz_v[:, 0:CO],
                                       op0=AL.mult, op1=AL.subtract)
                tm = sc.tile([P, CO], F32, tag=f"tm{t}")
                v.tensor_tensor(out=tm, in0=tmr, in1=R[:, 0:CO], op=AL.mult)
                if affine:
                    tpp = z_v[:, CO:V]
                else:
                    tpp = sc.tile([P, CO], F32, tag=f"tpp{t}")
                    v.tensor_tensor(out=tpp, in0=rsz_v[:, CO:V],
                                    in1=R[:, CO:V], op=AL.mult)
                A1 = sc.tile([P, CO], F32, tag=f"a1{t}")
                v.tensor_tensor(out=A1, in0=tm, in1=PRX, op=AL.add)
                A2 = sc.tile([P, CO], F32, tag=f"a2{t}")
                v.tensor_tensor(out=A2, in0=A1, in1=tpp, op=AL.add)
                FNT = sc.tile([P, CO], F32, tag=f"fnt{t}")
                v.tensor_scalar(out=FNT, in0=F8, scalar1=NTDF,
                                scalar2=NTt[:, 0:1], op0=AL.mult,
                                op1=AL.add)
                YS = sc.tile([P, CO], F32, tag=f"ys{t}")
                v.tensor_tensor(out=YS, in0=A2, in1=FNT, op=AL.add)
                AC2 = sc.tile([P, 2], F32, tag=f"ac2{t}")
                Y = sc.tile([P, CO], F32, tag=f"y{t}")
                v.scalar_tensor_tensor(out=Y, in0=YS, scalar=1.0,
                                       in1=DIB[:, 0:CO], op0=AL.bypass,
                                       op1=AL.mult, accum_out=AC2[:, 0:1])
                FY = sc.tile([P, CO], F32, tag=f"fy{t}")
                v.scalar_tensor_tensor(out=FY, in0=Y, scalar=1.0, in1=F8,
                                       op0=AL.bypass, op1=AL.mult,
                                       accum_out=AC2[:, 1:2])
                S12 = ps.tile([P, 2], F32, tag="s12")
                pe.matmul(S12, ONES, AC2)
                if mid_cb is not None:
                    mid_cb()
                AB3 = sc.tile([P, 3], F32, tag=f"ab3{t}")
                q4 = sc.tile([P, 1], F32, tag=f"q4{t}")
                v.tensor_tensor(out=q4, in0=S12[:, 1:2], in1=VUSS[:, 0:1],
                                op=AL.mult)
                t1 = sc.tile([P, 1], F32, tag=f"t1{t}")
                v.tensor_tensor(out=t1, in0=S12[:, 0:1], in1=M22,
                                op=AL.mult)
                v.tensor_tensor(out=AB3[:, 0:1], in0=t1, in1=q4,
                                op=AL.subtract)
                q3 = sc.tile([P, 1], F32, tag=f"q3{t}")
                v.tensor_tensor(out=q3, in0=S12[:, 0:1], in1=VUSS[:, 0:1],
                                op=AL.mult)
                t2 = sc.tile([P, 1], F32, tag=f"t2{t}")
                v.tensor_tensor(out=t2, in0=S12[:, 1:2], in1=M11,
                                op=AL.mult)
                v.tensor_tensor(out=AB3[:, 1:2], in0=t2, in1=q3,
                                op=AL.subtract)
                v.tensor_copy(AB3[:, 2:3], AB3[:, 1:2])
                c8 = sc.tile([P, CO], F32, tag=f"c8{t}")
                v.tensor_scalar(out=c8, in0=F8, scalar1=AB3[:, 1:2],
                                scalar2=AB3[:, 0:1], op0=AL.mult,
                                op1=AL.add)
                m1 = sc.tile([P, CO], F32, tag=f"m1{t}")
                v.tensor_tensor(out=m1, in0=DIDETI, in1=c8, op=AL.mult)
                v.tensor_tensor(out=DXt, in0=Y, in1=m1, op=AL.add)
                # ds vector lanes
                v.tensor_scalar(out=DSZ[:, 0:CO], in0=DXt, scalar1=PHI,
                                scalar2=None, op0=AL.subtract)
                v.tensor_scalar(out=DSZ[:, CO:V], in0=DXt, scalar1=-1.0,
                                scalar2=None, op0=AL.mult)
                # scalar-constraint lanes: [ABa|ABb|ABb] * [nP1|nP2|P2]
                # - RPs  and  [ABa|ABb|ABb] * [negDETI|P3|P4] + NT
                DS3t = sc.tile([P, 3], F32, tag=f"ds3t{t}")
                g.tensor_tensor(out=DS3t, in0=AB3, in1=PP3, op=AL.mult)
                g.tensor_tensor(out=DSZ[:, V:NS], in0=DS3t, in1=RPs,
                                op=AL.subtract)
                NZ3t = sc.tile([P, 3], F32, tag=f"nz3t{t}")
                g.tensor_tensor(out=NZ3t, in0=AB3, in1=PN3, op=AL.mult)
                g.tensor_tensor(out=DSZ[:, NS + V:2 * NS], in0=NZ3t,
                                in1=NTt, op=AL.add)
                # ndz vector lanes
                if affine:
                    uv = sc.tile([P, V], F32, tag=f"uv{t}")
                    v.tensor_tensor(out=uv, in0=DSZ[:, 0:V], in1=s_v,
                                    op=AL.add)
                    v.tensor_tensor(out=DSZ[:, NS:NS + V], in0=W16,
                                    in1=uv, op=AL.mult)
                else:
                    uv = sc.tile([P, V], F32, tag=f"uv{t}")
                    v.tensor_tensor(out=uv, in0=z_v, in1=DSZ[:, 0:V],
                                    op=AL.mult)
                    vv = sc.tile([P, V], F32, tag=f"vv{t}")
                    v.tensor_tensor(out=vv, in0=uv, in1=rsz_v, op=AL.add)
                    v.tensor_tensor(out=DSZ[:, NS:NS + V], in0=vv,
                                    in1=R[:, 0:V], op=AL.mult)
                # fraction-to-boundary ratios: DSZ * [-1/s | 1/z]
                Q38 = sc.tile([P, 2 * NS], F32, tag=f"q38{t}")
                v.tensor_tensor(out=Q38, in0=DSZ, in1=RS38, op=AL.mult)
                QP2 = sc.tile([P, 1], F32, tag=f"qp2{t}")
                v.reduce_max(QP2, Q38, axis=AX)
                # corrector cross terms + mu_aff sums (affine only)
                MAC = PQ16 = PQS = None
                if mu_acc:
                    MAC = sc.tile([P, 3], F32, tag="mac")
                    TZo = sc.tile([P, V], F32, tag="tzo")
                    v.scalar_tensor_tensor(out=TZo, in0=DSZ[:, 0:V],
                                           scalar=1.0, in1=z_v,
                                           op0=AL.bypass, op1=AL.mult,
                                           accum_out=MAC[:, 0:1])
                    USo = sc.tile([P, V], F32, tag="uso")
                    v.scalar_tensor_tensor(out=USo, in0=DSZ[:, NS:NS + V],
                                           scalar=-1.0, in1=s_v,
                                           op0=AL.mult, op1=AL.mult,
                                           accum_out=MAC[:, 1:2])
                    PQ16 = sc.tile([P, V], F32, tag="pq16")
                    v.scalar_tensor_tensor(out=PQ16, in0=DSZ[:, 0:V],
                                           scalar=-1.0,
                                           in1=DSZ[:, NS:NS + V],
                                           op0=AL.mult, op1=AL.mult,
                                           accum_out=MAC[:, 2:3])
                    PQS = sc.tile([P, 3], F32, tag="pqs")
                    v.scalar_tensor_tensor(out=PQS, in0=DSZ[:, V:NS],
                                           scalar=-1.0,
                                           in1=DSZ[:, NS + V:2 * NS],
                                           op0=AL.mult, op1=AL.mult)
                return QP2, MAC, PQ16, PQS

            TQP = psq.tile([1, P + 1], F32, tag="b4")
            v.memset(TQP[0:1, P:P + 1], 1.0)

            for it in range(ITERS):
                # ---------------- stage A ----------------
                R = sc.tile([P, 2 * NS], F32, tag="R")
                v.reciprocal(R, SZ)
                # act: per-partition scalars available early
                AINV = sc.tile([P, 1], F32, tag="AINV")
                a.activation(AINV, SZ[:, V:V + 1], ACT.Copy,
                             scale=R[:, NS + V:NS + V + 1])
                RPs = sc.tile([P, 3], F32, tag="RPs")
                a.activation(RPs, RF, ACT.Copy, scale=PHI)
                # gpsimd early chain
                SZPs = sc.tile([P, 3], F32, tag="SZPs")
                g.tensor_tensor(out=SZPs, in0=s_s, in1=z_s, op=AL.mult)
                Ws3 = sc.tile([P, 3], F32, tag="Ws3")
                g.tensor_tensor(out=Ws3, in0=z_s, in1=R[:, V:NS],
                                op=AL.mult)
                Wf = Ws3[:, 1:3]
                RS38 = sc.tile([P, 2 * NS], F32, tag="RS38")
                v.tensor_tensor(out=RS38, in0=R, in1=SGN38, op=AL.mult)
                PRX = sc.tile([P, CO], F32, tag="PRX")
                v.tensor_scalar(out=PRX, in0=RX0, scalar1=NPHI,
                                scalar2=None, op0=AL.mult)
                # vector: D and sums
                W16 = sc.tile([P, V], F32, tag="W16")
                v.tensor_tensor(out=W16, in0=z_v, in1=R[:, 0:V], op=AL.mult)
                DtB = sc.tile([P, CO + 1], F32, tag="DtB")
                v.scalar_tensor_tensor(out=DtB[:, 0:CO], in0=W16[:, 0:CO],
                                       scalar=EPS, in1=W16[:, CO:V],
                                       op0=AL.add, op1=AL.add)
                g.tensor_tensor(out=DtB[:, CO:CO + 1], in0=Wf[:, 0:1],
                                in1=Wf[:, 1:2], op=AL.add)
                DIB = sc.tile([P, CO + 1], F32, tag="DIB")
                v.reciprocal(DIB, DtB)
                ACC3 = sc.tile([P, 3], F32, tag="ACC3")
                DIFt = sc.tile([P, CO], F32, tag="DIFt")
                v.scalar_tensor_tensor(out=DIFt, in0=DIB[:, 0:CO],
                                       scalar=1.0, in1=F8, op0=AL.bypass,
                                       op1=AL.mult, accum_out=ACC3[:, 0:1])
                v.reduce_sum(ACC3[:, 1:2], DIB[:, 0:CO], axis=AX)
                SZPv = sc.tile([P, V], F32, tag="SZPv")
                v.scalar_tensor_tensor(out=SZPv, in0=s_v, scalar=1.0,
                                       in1=z_v, op0=AL.bypass, op1=AL.mult,
                                       accum_out=ACC3[:, 2:3])
                VUS = ps.tile([P, 3], F32, tag="b0")
                pe.matmul(VUS, ONES, ACC3)
                VUSS = sc.tile([P, 3], F32, tag="VUSS")
                a.copy(VUSS, VUS)
                sv2a = sc.tile([P, 1], F32, tag="sv2a")
                a.activation(sv2a, VUS[:, 0:1], ACT.Square)
                M22 = sc.tile([P, 1], F32, tag="M22")
                v.tensor_tensor(out=M22, in0=DIB[:, CO:CO + 1],
                                in1=VUS[:, 0:1], op=AL.add)
                M11 = sc.tile([P, 1], F32, tag="M11")
                v.tensor_tensor(out=M11, in0=AINV, in1=VUS[:, 1:2],
                                op=AL.add)
                # vector: direction-a head (independent of VUS)
                NTa = sc.tile([P, 3], F32, tag="nta")
                DSZa = sc.tile([P, 2 * NS], F32, tag="DSZa")
                DXa = sc.tile([P, CO], F32, tag="DXa")
                DIDETI = sc.tile([P, CO], F32, tag="DIDETI")
                PP3 = sc.tile([P, 3], F32, tag="PP3")  # [nP1|nP2|P2]
                PN3 = sc.tile([P, 3], F32, tag="PN3")  # [negDETI|P3|P4]

                RMUv = sc.tile([1, 1], F32, tag="RMUv")
                nMoMv = sc.tile([1, 1], F32, tag="nMoMv")

                def mid_a():
                    # negdet = Sv^2 - M11*M22 ; negDETI = 1/negdet
                    # (fills the S12 PE round trip)
                    detA = sc.tile([P, 1], F32, tag="detA")
                    v.tensor_tensor(out=detA, in0=M11, in1=M22, op=AL.mult)
                    negdet = sc.tile([P, 1], F32, tag="negdet")
                    v.tensor_tensor(out=negdet, in0=sv2a, in1=detA,
                                    op=AL.subtract)
                    v.reciprocal(PN3[:, 0:1], negdet)
                    v.tensor_scalar(out=DIDETI, in0=DIB[:, 0:CO],
                                    scalar1=PN3[:, 0:1], scalar2=None,
                                    op0=AL.mult)
                    a.activation(PP3[:, 0:1], AINV, ACT.Copy,
                                 scale=PN3[:, 0:1])
                    a.activation(PP3[:, 1:2], DIB[:, CO:CO + 1], ACT.Copy,
                                 scale=PN3[:, 0:1])
                    a.mul(PP3[:, 2:3], PP3[:, 1:2], -1.0)
                    g.tensor_tensor(out=PN3[:, 1:2], in0=Wf[:, 0:1],
                                    in1=PP3[:, 1:2], op=AL.mult)
                    g.tensor_tensor(out=PN3[:, 2:3], in0=Wf[:, 1:2],
                                    in1=PP3[:, 2:3], op=AL.mult)
                    # (1,1) mu prep on partition 0 (vector lanes only)
                    v.reciprocal(RMUv, VUS[0:1, 2:3])
                    v.tensor_scalar(out=nMoMv, in0=VUS[0:1, 2:3],
                                    scalar1=-1.0 / M_CONST, scalar2=None,
                                    op0=AL.mult)

                QP2a, MAC, PQ16, PQS = direction(
                    "a", SZPv, SZPs, R, RS38, RPs, NTa, DIB, DIDETI,
                    VUSS, M22, M11, PP3, PN3, W16, Ws3, PRX, DSZa, DXa,
                    True, mid_cb=mid_a)

                TQ = TQP
                pe.transpose(TQ[0:1, 0:P], QP2a, IDENT)
                TUP = ps.tile([P, 3], F32, tag="b2")
                pe.matmul(TUP, ONES, MAC)
                TUPS = sc.tile([P, 3], F32, tag="TUPS")
                a.copy(TUPS, TUP)

                # (1,1) block: a_aff, mu_aff (vector lanes), sigma
                qm = sc.tile([1, 1], F32, tag="qm")
                v.reduce_max(qm, TQ, axis=AX)
                aaf = sc.tile([1, 1], F32, tag="aaf")
                v.reciprocal(aaf, qm)
                C1 = sc.tile([1, 1], F32, tag="C1")
                v.tensor_tensor(out=C1, in0=TUPS[0:1, 0:1],
                                in1=TUPS[0:1, 1:2], op=AL.add)
                h1 = sc.tile([1, 1], F32, tag="h1")
                v.scalar_tensor_tensor(out=h1, in0=TUP[0:1, 2:3],
                                       scalar=aaf, in1=C1, op0=AL.mult,
                                       op1=AL.add)
                h2 = sc.tile([1, 1], F32, tag="h2")
                v.scalar_tensor_tensor(out=h2, in0=h1, scalar=aaf,
                                       in1=VUSS[0:1, 2:3], op0=AL.mult,
                                       op1=AL.add)
                rat = sc.tile([1, 1], F32, tag="rat")
                v.tensor_tensor(out=rat, in0=h2, in1=RMUv, op=AL.mult)
                r2 = sc.tile([1, 1], F32, tag="r2")
                v.tensor_tensor(out=r2, in0=rat, in1=rat, op=AL.mult)
                nsm1 = sc.tile([1, 1], mybir.dt.bfloat16, tag="nsm1")
                v.scalar_tensor_tensor(out=nsm1, in0=r2, scalar=rat,
                                       in1=nMoMv, op0=AL.mult, op1=AL.mult)
                NSB = ps.tile([P, 1], F32, tag="b3")
                pe.matmul(NSB, ONESB, nsm1)

                # corrector rsz
                RC16 = sc.tile([P, V], F32, tag="RC16")
                v.scalar_tensor_tensor(out=RC16, in0=PQ16, scalar=NSB,
                                       in1=SZPv, op0=AL.add, op1=AL.add)
                RCs = sc.tile([P, 3], F32, tag="RCs")
                v.scalar_tensor_tensor(out=RCs, in0=PQS, scalar=NSB,
                                       in1=SZPs, op0=AL.add, op1=AL.add)

                # ---------------- corrector direction ----------------
                NTc = sc.tile([P, 3], F32, tag="ntc")
                DSZc = sc.tile([P, 2 * NS], F32, tag="DSZc")
                DXc = sc.tile([P, CO], F32, tag="DXc")
                QP2c, _, _, _ = direction(
                    "c", RC16, RCs, R, RS38, RPs, NTc, DIB, DIDETI,
                    VUSS, M22, M11, PP3, PN3, W16, Ws3, PRX, DSZc, DXc,
                    False)

                TQc = TQP
                pe.transpose(TQc[0:1, 0:P], QP2c, IDENT)
                qm2 = sc.tile([1, 1], F32, tag="qm2")
                v.reduce_max(qm2, TQc, axis=AX)
                acr = sc.tile([1, 1], F32, tag="acr")
                v.reciprocal(acr, qm2)
                AB3 = sc.tile([1, 3], F32, tag="ab3")
                v.scalar_tensor_tensor(out=AB3, in0=CA3, scalar=acr,
                                       in1=CB3, op0=AL.mult, op1=AL.add)
                ABC = ps.tile([P, 3], F32, tag="b2")
                pe.matmul(ABC, ONES[0:1, :], AB3)

                # ---------------- updates ----------------
                v.scalar_tensor_tensor(out=SZ[:, 0:NS], in0=DSZc[:, 0:NS],
                                       scalar=ABC[:, 0:1], in1=SZ[:, 0:NS],
                                       op0=AL.mult, op1=AL.add)
                v.scalar_tensor_tensor(out=SZ[:, NS:2 * NS],
                                       in0=DSZc[:, NS:2 * NS],
                                       scalar=ABC[:, 1:2],
                                       in1=SZ[:, NS:2 * NS],
                                       op0=AL.mult, op1=AL.add)
                v.tensor_tensor(out=PHI, in0=PHI, in1=ABC[:, 2:3],
                                op=AL.mult)
                v.tensor_scalar(out=NPHI, in0=PHI, scalar1=-1.0,
                                scalar2=None, op0=AL.mult)
                v.scalar_tensor_tensor(out=XT, in0=DXc, scalar=ABC[:, 0:1],
                                       in1=XT, op0=AL.mult, op1=AL.add)

            # ---------------- end projection ----------------
            AF = sc.tile([P, 4], F32, tag="AF")
            fxv = sc.tile([P, CO], F32, tag="fxv")
            v.scalar_tensor_tensor(out=fxv, in0=XT, scalar=1.0, in1=F8,
                                   op0=AL.bypass, op1=AL.mult,
                                   accum_out=AF[:, 0:1])
            XTc = sc.tile([P, CO], F32, tag="XTc")
            a.activation(XTc, XT, ACT.Copy, accum_out=AF[:, 1:2])
            R2 = sc.tile([P, 2 * NS], F32, tag="R")
            v.reciprocal(R2, SZ)
            W2 = sc.tile([P, V], F32, tag="W16")
            v.tensor_tensor(out=W2, in0=z_v, in1=R2[:, 0:V], op=AL.mult)
            D2 = sc.tile([P, CO], F32, tag="D2")
            v.scalar_tensor_tensor(out=D2, in0=W2[:, 0:CO], scalar=EPS,
                                   in1=W2[:, CO:V], op0=AL.add, op1=AL.add)
            DI2r = sc.tile([P, CO], F32, tag="DI2r")
            v.reciprocal(DI2r, D2)
            DI2 = sc.tile([P, CO], F32, tag="DI2")
            v.tensor_scalar(out=DI2, in0=DI2r, scalar1=1e-4, scalar2=None,
                            op0=AL.max)
            DIF2 = sc.tile([P, CO], F32, tag="DIF2")
            v.scalar_tensor_tensor(out=DIF2, in0=DI2, scalar=1.0, in1=F8,
                                   op0=AL.bypass, op1=AL.mult,
                                   accum_out=AF[:, 2:3])
            DI2c = sc.tile([P, CO], F32, tag="DI2c")
            a.activation(DI2c, DI2, ACT.Copy, accum_out=AF[:, 3:4])
            VF = ps.tile([P, 4], F32, tag="b3")     # [Fx | Sx | Sv2 | Su2]
            pe.matmul(VF, ONES, AF)
            VFS = sc.tile([P, 4], F32, tag="VFS")
            a.copy(VFS, VF)

            GT3 = sc.tile([P, 3], F32, tag="GT3")
            v.tensor_tensor(out=GT3, in0=z_s, in1=s_s, op=AL.is_gt)
            gf = sc.tile([P, 1], F32, tag="gf")
            v.tensor_tensor(out=gf, in0=GT3[:, 1:2], in1=GT3[:, 2:3],
                            op=AL.max)
            d0 = sc.tile([P, 1], F32, tag="d0")
            v.scalar_tensor_tensor(out=d0, in0=VF[:, 1:2], scalar=-C_CAP,
                                   in1=s_s[:, 0:1], op0=AL.add, op1=AL.add)
            ta = sc.tile([P, 1], F32, tag="ta")
            g.tensor_tensor(out=ta, in0=VFS[:, 0:1], in1=s_s[:, 1:2],
                            op=AL.add)
            dfa = sc.tile([P, 1], F32, tag="dfa")
            g.tensor_tensor(out=dfa, in0=ta, in1=RF[:, 2:3], op=AL.subtract)
            tb = sc.tile([P, 1], F32, tag="tb")
            g.tensor_tensor(out=tb, in0=s_s[:, 2:3], in1=VFS[:, 0:1],
                            op=AL.subtract)
            dfb = sc.tile([P, 1], F32, tag="dfb")
            g.tensor_tensor(out=dfb, in0=tb, in1=RF[:, 1:2], op=AL.subtract)
            ua = sc.tile([P, 1], F32, tag="ua")
            g.tensor_tensor(out=ua, in0=GT3[:, 1:2], in1=dfa, op=AL.mult)
            ub = sc.tile([P, 1], F32, tag="ub")
            g.tensor_tensor(out=ub, in0=GT3[:, 2:3], in1=dfb, op=AL.mult)
            df = sc.tile([P, 1], F32, tag="df")
            g.tensor_tensor(out=df, in0=ua, in1=ub, op=AL.subtract)
            gdf = sc.tile([P, 1], F32, tag="gdf")
            g.tensor_tensor(out=gdf, in0=gf, in1=df, op=AL.mult)
            num0 = sc.tile([P, 1], F32, tag="num0")
            v.tensor_tensor(out=num0, in0=d0, in1=gdf, op=AL.subtract)
            gsv = sc.tile([P, 1], F32, tag="gsv")
            v.scalar_tensor_tensor(out=gsv, in0=VFS[:, 2:3], scalar=-1.0,
                                   in1=gf, op0=AL.mult, op1=AL.mult)
            den0 = sc.tile([P, 1], F32, tag="den0")
            v.tensor_tensor(out=den0, in0=VFS[:, 3:4], in1=gsv, op=AL.add)
            dd = sc.tile([P, 1], F32, tag="dd")
            v.tensor_scalar(out=dd, in0=den0, scalar1=den0, scalar2=TINY,
                            op0=AL.mult, op1=AL.add)
            rdd = sc.tile([P, 1], F32, tag="rdd")
            v.reciprocal(rdd, dd)
            nu = sc.tile([P, 1], F32, tag="nu")
            v.tensor_tensor(out=nu, in0=num0, in1=den0, op=AL.mult)
            v0a = sc.tile([P, 1], F32, tag="v0a")
            v.tensor_tensor(out=v0a, in0=nu, in1=rdd, op=AL.mult)
            v0 = sc.tile([P, 1], F32, tag="v0")
            v.tensor_tensor(out=v0, in0=GT3[:, 0:1], in1=v0a, op=AL.mult)
            sv2d = sc.tile([P, 1], F32, tag="sv2d")
            v.tensor_scalar(out=sv2d, in0=VFS[:, 2:3], scalar1=VFS[:, 2:3],
                            scalar2=TINY, op0=AL.mult, op1=AL.add)
            rsv = sc.tile([P, 1], F32, tag="rsv")
            v.reciprocal(rsv, sv2d)
            u1n = sc.tile([P, 1], F32, tag="u1n")
            g.tensor_tensor(out=u1n, in0=df, in1=VFS[:, 2:3], op=AL.mult)
            v1u = sc.tile([P, 1], F32, tag="v1u")
            v.tensor_tensor(out=v1u, in0=u1n, in1=rsv, op=AL.mult)
            w1 = sc.tile([P, 1], F32, tag="w1")
            v.tensor_tensor(out=w1, in0=v1u, in1=gf, op=AL.mult)
            omgf = sc.tile([P, 1], F32, tag="omgf")
            v.tensor_scalar(out=omgf, in0=gf, scalar1=-1.0, scalar2=1.0,
                            op0=AL.mult, op1=AL.add)
            w3 = sc.tile([P, 1], F32, tag="w3")
            v.tensor_tensor(out=w3, in0=omgf, in1=v0, op=AL.mult)
            v1 = sc.tile([P, 1], F32, tag="v1")
            v.tensor_tensor(out=v1, in0=w1, in1=w3, op=AL.add)
            bee = sc.tile([P, 1], F32, tag="bee")
            v.tensor_tensor(out=bee, in0=v1, in1=v0, op=AL.subtract)
            corr = sc.tile([P, CO], F32, tag="corr")
            v.tensor_scalar(out=corr, in0=F8, scalar1=bee, scalar2=v0,
                            op0=AL.mult, op1=AL.add)
            mcor = sc.tile([P, CO], F32, tag="mcor")
            v.tensor_tensor(out=mcor, in0=DI2, in1=corr, op=AL.mult)
            v.tensor_tensor(out=XT, in0=XT, in1=mcor, op=AL.subtract)
            v.tensor_scalar(out=XT, in0=XT, scalar1=0.0, scalar2=1.0,
                            op0=AL.max, op1=AL.min)
            nc.sync.dma_start(out=o_ap, in_=XT)

    return nc


_CACHE: dict = {}


def _get_nc():
    if "nc" not in _CACHE:
        nc = bacc.Bacc(None, target_bir_lowering=False)
        _build(nc)
        nc.finalize()
        _CACHE["nc"] = nc
    return _CACHE["nc"]


def kernel(x: np.ndarray, indices_male: np.ndarray) -> np.ndarray:
    nc = _get_nc()
    base = {
        "x": np.ascontiguousarray(x, dtype=np.float32),
        "ind": np.ascontiguousarray(indices_male, dtype=np.int32),
    }
    in_maps = [dict(base) for _ in range(8)]
    res = run_bass_kernel_spmd(nc, in_maps, core_ids=list(range(8)))
    return np.asarray(res.results[0]["out"], dtype=np.float32)


if __name__ == "__main__":
    rng = np.random.default_rng(0)
    x = rng.standard_normal((1, N)).astype(np.float32)
    f = (np.arange(N) % 2).astype(np.int32)
    out = kernel(x, f)
    print("out", out.shape, out.dtype, out[0, :6], out.sum())


# revision 3
# speedup vs baseline: 4.3315x; 1.0565x over previous
"""Trainium2 Bass kernel for nn_CapLayerLP — direct dual-threshold solve,
fused: all five 1-D bisections run concurrently in one 24-step loop.

Solves x = clip((v - lam - mu*f)/eps, 0, 1). Lane layout of the packed
bisection state (P,5): [lamA | tau_m@BH-case? no: tau_m@BL | tau_m@BH |
tau_f@(C-BL) | tau_f@(C-BH)]. Branch A (fairness inactive) uses lamA;
branch B picks the male/female pair by which fairness bound the
branch-A solution violates. Masked solves use pre-masked inputs
XM = f*v, XF = (1-f)*v (tau > 0 always, so masked-out zeros clip to 0).
Bisection widths are compile-time constants. Validated in fp32 numpy:
worst rel err 1.1e-3 over 220 random instances (24 steps).
"""
import numpy as np

import concourse.bass as bass
import concourse.bacc as bacc
import concourse.tile as tile
from concourse import mybir
from concourse.bass_utils import run_bass_kernel_spmd

AL = mybir.AluOpType
F32 = mybir.dt.float32
AX = mybir.AxisListType.X

N = 1024
P = 128
CO = N // P            # 8
C_CAP = 10.0
EPSI = 1e4             # 1/eps
import os
STEPS = int(os.environ.get("KB_STEPS", "24"))


def _build(nc: bass.Bass):
    x_d = nc.dram_tensor("x", [1, N], F32, kind="ExternalInput")
    f_d = nc.dram_tensor("ind", [N], mybir.dt.int32, kind="ExternalInput")
    out_d = nc.dram_tensor("out", [1, N], F32, kind="ExternalOutput")

    x_ap = x_d[:, :].rearrange("a (p c) -> a p c", p=P)[0]
    f_ap = f_d[:].rearrange("(p c) -> p c", p=P)
    o_ap = out_d[:, :].rearrange("a (p c) -> a p c", p=P)[0]

    with tile.TileContext(nc) as tc:
        with (
            tc.tile_pool(name="const", bufs=1) as cns,
            tc.tile_pool(name="state", bufs=1) as st,
            tc.tile_pool(name="scr", bufs=2) as sc,
            tc.tile_pool(name="psum", bufs=2, space="PSUM") as ps,
        ):
            v = nc.vector
            g = nc.gpsimd
            a = nc.scalar
            pe = nc.tensor

            ONES = cns.tile([P, P], F32)
            g.memset(ONES, 1.0)
            F8 = cns.tile([P, CO], F32)
            nc.gpsimd.dma_start(out=F8, in_=f_ap)   # int32 -> f32 cast
            XIN = st.tile([P, CO], F32)
            nc.sync.dma_start(out=XIN, in_=x_ap)
            ONE8 = cns.tile([P, CO], F32)
            v.memset(ONE8, 1.0)
            OMF8 = cns.tile([P, CO], F32)
            v.tensor_scalar(out=OMF8, in0=F8, scalar1=-1.0, scalar2=1.0,
                            op0=AL.mult, op1=AL.add)
            XM = cns.tile([P, CO], F32)
            v.tensor_tensor(out=XM, in0=XIN, in1=F8, op=AL.mult)
            XF = cns.tile([P, CO], F32)
            v.tensor_tensor(out=XF, in0=XIN, in1=OMF8, op=AL.mult)

            # fairness bounds + packed targets
            facc = sc.tile([P, 1], F32, tag="facc")
            v.reduce_sum(facc, F8, axis=AX)
            NMp = ps.tile([P, 1], F32, tag="b0")
            pe.matmul(NMp, ONES, facc)
            TGT5 = cns.tile([P, 5], F32)   # [C | BL | BH | C-BL | C-BH]
            v.memset(TGT5[:, 0:1], C_CAP)
            v.tensor_scalar(out=TGT5[:, 1:2], in0=NMp, scalar1=C_CAP / N,
                            scalar2=None, op0=AL.mult)
            v.tensor_scalar(out=TGT5[:, 2:3], in0=TGT5[:, 1:2],
                            scalar1=1.0, scalar2=None, op0=AL.add)
            v.tensor_scalar(out=TGT5[:, 3:4], in0=TGT5[:, 1:2],
                            scalar1=-1.0, scalar2=C_CAP, op0=AL.mult,
                            op1=AL.add)
            v.tensor_scalar(out=TGT5[:, 4:5], in0=TGT5[:, 2:3],
                            scalar1=-1.0, scalar2=C_CAP, op0=AL.mult,
                            op1=AL.add)

            LO5 = st.tile([P, 5], F32)
            v.memset(LO5, 0.0)
            TAU5 = st.tile([P, 5], F32)
            v.memset(TAU5, 4.0)
            SRC = [XIN, XM, XM, XF, XF]

            for k in range(STEPS):
                AC5 = sc.tile([P, 5], F32, tag="AC5")
                for j in range(5):
                    t = sc.tile([P, CO], F32, tag=f"t{j}")
                    v.tensor_scalar(out=t, in0=SRC[j],
                                    scalar1=TAU5[:, j:j + 1], scalar2=EPSI,
                                    op0=AL.subtract, op1=AL.mult)
                    c = sc.tile([P, CO], F32, tag=f"c{j}")
                    v.scalar_tensor_tensor(out=c, in0=t, scalar=0.0,
                                           in1=ONE8, op0=AL.max,
                                           op1=AL.min,
                                           accum_out=AC5[:, j:j + 1])
                S5 = ps.tile([P, 5], F32, tag="b1")
                pe.matmul(S5, ONES, AC5)
                pred5 = sc.tile([P, 5], F32, tag="pred5")
                v.tensor_tensor(out=pred5, in0=S5, in1=TGT5, op=AL.is_gt)
                dW5 = sc.tile([P, 5], F32, tag="dW5")
                v.tensor_scalar(out=dW5, in0=pred5, scalar1=4.0 / 2.0**k,
                                scalar2=None, op0=AL.mult)
                v.tensor_tensor(out=LO5, in0=LO5, in1=dW5, op=AL.add)
                v.tensor_scalar(out=TAU5, in0=LO5,
                                scalar1=4.0 / 2.0**(k + 1), scalar2=None,
                                op0=AL.add)

            # branch-A solution + male mass
            tA = sc.tile([P, CO], F32, tag="tA")
            v.tensor_scalar(out=tA, in0=XIN, scalar1=TAU5[:, 0:1],
                            scalar2=EPSI, op0=AL.subtract, op1=AL.mult)
            XA = st.tile([P, CO], F32)
            v.scalar_tensor_tensor(out=XA, in0=tA, scalar=0.0, in1=ONE8,
                                   op0=AL.max, op1=AL.min)
            AFm = sc.tile([P, 1], F32, tag="AFm")
            fmv = sc.tile([P, CO], F32, tag="fmv")
            v.scalar_tensor_tensor(out=fmv, in0=XA, scalar=1.0, in1=F8,
                                   op0=AL.bypass, op1=AL.mult,
                                   accum_out=AFm)
            FM = ps.tile([P, 1], F32, tag="b2")
            pe.matmul(FM, ONES, AFm)
            PREDH = sc.tile([P, 1], F32, tag="PREDH")
            v.tensor_tensor(out=PREDH, in0=FM, in1=TGT5[:, 2:3],
                            op=AL.is_gt)
            PREDL = sc.tile([P, 1], F32, tag="PREDL")
            v.tensor_tensor(out=PREDL, in0=FM, in1=TGT5[:, 1:2],
                            op=AL.is_lt)
            USEB = sc.tile([P, 1], F32, tag="USEB")
            v.tensor_tensor(out=USEB, in0=PREDH, in1=PREDL, op=AL.add)
            # pick tau_m, tau_f by violated bound (PREDH selects @BH pair)
            dtm = sc.tile([P, 1], F32, tag="dtm")
            v.tensor_tensor(out=dtm, in0=TAU5[:, 2:3], in1=TAU5[:, 1:2],
                            op=AL.subtract)
            TM = sc.tile([P, 1], F32, tag="TM")
            v.scalar_tensor_tensor(out=TM, in0=dtm, scalar=PREDH,
                                   in1=TAU5[:, 1:2], op0=AL.mult,
                                   op1=AL.add)
            dtf = sc.tile([P, 1], F32, tag="dtf")
            v.tensor_tensor(out=dtf, in0=TAU5[:, 4:5], in1=TAU5[:, 3:4],
                            op=AL.subtract)
            TF = sc.tile([P, 1], F32, tag="TF")
            v.scalar_tensor_tensor(out=TF, in0=dtf, scalar=PREDH,
                                   in1=TAU5[:, 3:4], op0=AL.mult,
                                   op1=AL.add)
            # branch-B x (masked sources already zero out the other group)
            tm2 = sc.tile([P, CO], F32, tag="tm2")
            v.tensor_scalar(out=tm2, in0=XM, scalar1=TM, scalar2=EPSI,
                            op0=AL.subtract, op1=AL.mult)
            xm = sc.tile([P, CO], F32, tag="xm")
            v.scalar_tensor_tensor(out=xm, in0=tm2, scalar=0.0, in1=ONE8,
                                   op0=AL.max, op1=AL.min)
            tf2 = sc.tile([P, CO], F32, tag="tf2")
            v.tensor_scalar(out=tf2, in0=XF, scalar1=TF, scalar2=EPSI,
                            op0=AL.subtract, op1=AL.mult)
            xf = sc.tile([P, CO], F32, tag="xf")
            v.scalar_tensor_tensor(out=xf, in0=tf2, scalar=0.0, in1=ONE8,
                                   op0=AL.max, op1=AL.min)
            XB = sc.tile([P, CO], F32, tag="XB")
            v.tensor_tensor(out=XB, in0=xm, in1=xf, op=AL.add)
            DBA = sc.tile([P, CO], F32, tag="DBA")
            v.tensor_tensor(out=DBA, in0=XB, in1=XA, op=AL.subtract)
            XOUT = st.tile([P, CO], F32)
            v.scalar_tensor_tensor(out=XOUT, in0=DBA, scalar=USEB, in1=XA,
                                   op0=AL.mult, op1=AL.add)
            nc.sync.dma_start(out=o_ap, in_=XOUT)

    return nc


_CACHE: dict = {}


def _get_nc():
    if "nc" not in _CACHE:
        nc = bacc.Bacc(None, target_bir_lowering=False)
        _build(nc)
        nc.finalize()
        _CACHE["nc"] = nc
    return _CACHE["nc"]


def kernel(x: np.ndarray, indices_male: np.ndarray) -> np.ndarray:
    nc = _get_nc()
    base = {
        "x": np.ascontiguousarray(x, dtype=np.float32),
        "ind": np.ascontiguousarray(indices_male, dtype=np.int32),
    }
    in_maps = [dict(base) for _ in range(8)]
    res = run_bass_kernel_spmd(nc, in_maps, core_ids=list(range(8)))
    return np.asarray(res.results[0]["out"], dtype=np.float32)


if __name__ == "__main__":
    rng = np.random.default_rng(0)
    x = rng.standard_normal((1, N)).astype(np.float32)
    f = (np.arange(N) % 2).astype(np.int32)
    out = kernel(x, f)
    print("out", out.shape, out.dtype, out[0, :6], out.sum())
